# revision 3
# baseline (speedup 1.0000x reference)
"""Multi-head self-attention on 8 Trainium2 NeuronCores.

Sharding: batch (2) x head-groups (4 groups of 4 heads) -> 8 cores.
Per core: x[b] @ wq/wk/wv column slices (256 ch), 4 heads of attention,
row-parallel wo -> partial [2048, 1024] output; host sums the 4 group
partials per batch.

Design. PE (tensor) is the binding engine: ~390k matmul cycles/core
(scores 2x-packed via tile_position run ~2x; PV/projections at 1
col/cycle) ~ 150-170us busy depending on pstate; ScalarE EXP is 128
ACTIVATEs of [128,1024] ~ 135us. So the schedule keeps PE continuously
fed (pstate ramps to 2.4GHz only after ~3us of gapless execution) and
hides everything else inside the EXP/PE stream:
  - Head-PAIR packing: qT/kT stored [128 part = headA(0:64)|headB(64:128),
    2048 t] bf16. Score matmuls are K=64 row-group pairs (tile_position
    (0,0)/(64,0)) -> ~2x score throughput.
  - xT is staged per 512-t window as 4 separate dram tensors/SBUF tiles
    (tile-granular deps), DMA'd in priority order wk,wq,xt0,xt1,wv,
    xt2,xt3,wo; the prologue q/k groups chase xt0 so the first EXP
    fires ~16us in (vs ~29us with monolithic xT).
  - Slot = (pair j, th 512-block of t1, chunk i of 128 t2): packed score
    pair -> one [128,1024] EXP ACTIVATE (both heads).
  - j=0 windows: PV lagged one full th-window (o PSUM banks), leaving
    window (0,0) free to pump the q/k/v projection stream.
  - j=1 windows: PV runs IN-window lagged 2 slots (o banks; window (1,0)
    doubles up PV(0,3)+PV(1,0) on p banks to absorb the transition in an
    otherwise idle window). attnT ranges therefore finish one window
    earlier, so outproj o_pairs stream inside (1,2)/(1,3) and the tail
    after the last EXP shrinks to ~2 PV slots + norms + a few o_pairs.
  - PSUM: s [128,1024]x2 (4 banks) + o A/B [128,512] (2) + p stream (2).
  - o tiles staged to SBUF right after the last PV (denom f32 + data
    bf16 copies) so softmax normalize (recip -> gpsimd broadcast -> mul)
    runs off the critical path; norms are issued at the START of the
    window after their stage so attnT writes always precede their
    outproj readers in issue order.
  - projections/outproj cut into ~2-matmul quanta pumped from a single
    carry-over stream into spare tensor cycles each slot.
  - 2-slot score lookahead: S(i+2) issues right after ACT(i).
  - everything bf16 except PSUM accum + denominators; y output bf16,
    host sums partials in f32.
"""

import sys

sys.path.insert(0, "/opt/trn_rl_repo")

import numpy as np
import ml_dtypes
import concourse.bass as bass
import concourse.mybir as mybir
import concourse.tile as tile
from concourse import bacc
from concourse.bass_utils import run_bass_kernel_spmd

B, T, D = 2, 2048, 1024
NH = 4  # heads per core
HD = 64  # head dim
CH = NH * HD  # 256 channels per core
KD = D // 128  # 8 k-ptiles
TP = T // 128  # 16 t2 chunks
W = 512  # t1 window width
NTH = T // W  # 4 th windows
VW = HD + 1  # 65: v columns + ones column
VROW = NH * VW  # 260
VPAD = TP * VROW + 64

F32 = mybir.dt.float32
BF16 = mybir.dt.bfloat16
EXP = mybir.ActivationFunctionType.Exp

_cached_nc = None


def _wlayout(w):
    """[G*128, C] -> [128, G*C] kd-major host relayout (contiguous DMA)."""
    g = w.shape[0] // 128
    return np.ascontiguousarray(
        w.reshape(g, 128, w.shape[1]).transpose(1, 0, 2).reshape(128, -1)
    )


def _build():
    nc = bacc.Bacc(None, target_bir_lowering=False)
    xts_d = [
        nc.dram_tensor(f"xt{tb}", [128, KD * W], BF16, kind="ExternalInput")
        for tb in range(NTH)
    ]
    wq = nc.dram_tensor("wq", [128, KD * CH], BF16, kind="ExternalInput")
    wk = nc.dram_tensor("wk", [128, KD * CH], BF16, kind="ExternalInput")
    wv = nc.dram_tensor("wv", [128, KD * CH], BF16, kind="ExternalInput")
    wo = nc.dram_tensor("wo", [128, 2 * D], BF16, kind="ExternalInput")
    y = nc.dram_tensor("y", [T, D], BF16, kind="ExternalOutput")

    with tile.TileContext(nc) as tc:
        with (
            tc.tile_pool(name="sb", bufs=1) as sb,
            tc.tile_pool(name="pep", bufs=20) as pep,
            tc.tile_pool(name="ostg", bufs=4) as ostg,
            tc.tile_pool(name="small", bufs=2) as small,
            tc.tile_pool(name="ysp", bufs=4) as ysp,
            tc.tile_pool(name="ps_s", bufs=2, space="PSUM") as ps_s,
            tc.tile_pool(name="ps_o", bufs=1, space="PSUM") as ps_o,
            tc.tile_pool(name="ps_p", bufs=2, space="PSUM") as ps_p,
        ):
            xts = [sb.tile([128, KD * W], BF16, name=f"xts{tb}") for tb in range(NTH)]
            wqt = sb.tile([128, KD * CH], BF16)
            wkt = sb.tile([128, KD * CH], BF16)
            wvt = sb.tile([128, KD * CH], BF16)
            wot = sb.tile([128, 2 * D], BF16)
            qT = [
                [sb.tile([64, T], BF16, name=f"qT{j}{p}") for p in range(2)]
                for j in range(2)
            ]
            kT = [
                [sb.tile([64, T], BF16, name=f"kT{j}{p}") for p in range(2)]
                for j in range(2)
            ]
            scr = sb.tile([128, W], BF16, name="scr")
            vt = sb.tile([128, VPAD], BF16)
            attnT = [sb.tile([128, T], BF16, name=f"attnT{j}") for j in range(2)]

            # --- input DMAs: one sync queue, priority order.  wk/wq lead
            # (prologue), xt0/xt1 next (window-0 scores + early stream),
            # wv before xt2/xt3 (v stream starts mid window 0), wo last. ---
            nc.sync.dma_start(wkt[:], wk[:])
            nc.sync.dma_start(xts[0][:], xts_d[0][:])
            nc.sync.dma_start(wqt[:], wq[:])
            nc.sync.dma_start(xts[1][:], xts_d[1][:])
            nc.sync.dma_start(wvt[:], wv[:])
            nc.sync.dma_start(xts[2][:], xts_d[2][:])
            nc.sync.dma_start(xts[3][:], xts_d[3][:])
            nc.sync.dma_start(wot[:], wo[:])
            # ones columns of vt (offsets 64 + 65*k) + 64-col pad tail
            nc.vector.memset(
                bass.AP(vt.tensor, HD, [[VPAD, 128], [VW, NH * TP]]), 1.0
            )
            nc.vector.memset(vt[:, TP * VROW : VPAD], 1.0)

            # ---------- work-quantum generators (proj / outproj) ----------
            def qk_group(j, dst, wsb, tb):
                ps = ps_p.tile([128, W], F32, tag="p", name="pps")
                for kd in range(KD):
                    nc.tensor.matmul(
                        ps[:],
                        wsb[:, kd * CH + j * 128 : kd * CH + j * 128 + 128],
                        xts[tb][:, kd * W : (kd + 1) * W],
                        start=(kd == 0),
                        stop=(kd == KD - 1),
                    )
                    if kd % 2 == 1:
                        yield
                for par in range(2):
                    nc.vector.tensor_copy(
                        dst[par][:, tb * W : (tb + 1) * W],
                        ps[par * 64 : (par + 1) * 64, :],
                    )

            def v_group(tp):
                tb, off = tp // 4, (tp % 4) * 128
                ps = ps_p.tile([128, W], F32, tag="p", name="vps")
                for kd in range(KD):
                    nc.tensor.matmul(
                        ps[:, 0:CH],
                        xts[tb][:, kd * W + off : kd * W + off + 128],
                        wvt[:, kd * CH : (kd + 1) * CH],
                        start=(kd == 0),
                        stop=(kd == KD - 1),
                    )
                    if kd % 4 == 3:
                        yield
                nc.vector.tensor_copy(
                    bass.AP(vt.tensor, tp * VROW, [[VPAD, 128], [VW, NH], [1, HD]]),
                    ps[:, 0:CH].rearrange("p (h c) -> p h c", h=NH),
                )

            def o_pair(tp, tags=(None, None), scalar_cast=False):
                # one full output row [128, 1024] per generator: two psum
                # accumulations, two casts, ONE y DMA (descriptor setup is
                # ~640ns regardless of size, so merging halves the sync
                # queue's descriptor load that paces the tail drain).
                yt = ysp.tile([128, 2 * W], BF16, tag="yt2", name="yt")
                for ob in range(2):
                    if tags[ob] is None:
                        ps = ps_p.tile([128, W], F32, tag="p", name="ops")
                    else:
                        ps = ps_o.tile([128, W], F32, tag=tags[ob], name="ops")
                    for j in range(2):
                        nc.tensor.matmul(
                            ps[:],
                            attnT[j][:, tp * 128 : tp * 128 + 128],
                            wot[:, j * D + ob * W : j * D + (ob + 1) * W],
                            start=(j == 0),
                            stop=(j == 1),
                        )
                    yield
                    if scalar_cast and ob == 1:
                        nc.scalar.copy(yt[:, ob * W : (ob + 1) * W], ps[:])
                    else:
                        nc.vector.tensor_copy(yt[:, ob * W : (ob + 1) * W], ps[:])
                nc.sync.dma_start(y[tp * 128 : (tp + 1) * 128, :], yt[:])

            # ---------- carry-over work stream ----------
            stream = []

            def pump(n):
                k = 0
                while k < n and stream:
                    try:
                        next(stream[0])
                        k += 1
                    except StopIteration:
                        stream.pop(0)

            def drain_stream():
                while stream:
                    try:
                        next(stream[0])
                    except StopIteration:
                        stream.pop(0)

            # ---------- attention machinery ----------
            pe_saved = {}
            o_tiles = {}
            staged = {}
            pending_norm = []

            s_tiles = {}

            def score_mm(j, th, i):
                s = ps_s.tile([128, 2 * W], F32, tag="s", name="s")
                s_tiles[(j, th, i)] = s
                for par in range(2):
                    nc.tensor.matmul(
                        s[:, par * W : (par + 1) * W],
                        kT[j][par][:, i * 128 : i * 128 + 128],
                        qT[j][par][:, th * W : (th + 1) * W],
                        start=True,
                        stop=True,
                    )

            def act_exp(j, th, i):
                s = s_tiles.pop((j, th, i))
                pe = pep.tile([128, 2 * W], BF16, tag="pe", name="pe")
                nc.scalar.activation(pe[:], s[:], EXP, scale=0.125)
                pe_saved[(j, th, i)] = pe

            def pv(j, th, i, banks="o"):
                if i == 0:
                    if banks == "p":
                        o_tiles[(j, th)] = [
                            ps_p.tile([128, W], F32, tag="p", name="oP")
                            for _ in range(2)
                        ]
                    else:
                        o_tiles[(j, th)] = [
                            ps_o.tile([128, W], F32, tag="oA", name="oA"),
                            ps_o.tile([128, W], F32, tag="oB", name="oB"),
                        ]
                ot = o_tiles[(j, th)]
                pe = pe_saved.pop((j, th, i))
                for par in range(2):
                    hh = 2 * j + par
                    nc.tensor.matmul(
                        ot[par][:],
                        vt[:, i * VROW + hh * VW : i * VROW + hh * VW + 128],
                        pe[:, par * W : (par + 1) * W],
                        start=(i == 0),
                        stop=(i == TP - 1),
                    )

            def stage_o(j, th):
                # free the o PSUM banks fast: denom (f32) + data (bf16)
                ot = o_tiles.pop((j, th))
                st = {}
                for par in range(2):
                    den = small.tile([1, W], F32, tag="den", name="den")
                    dat = ostg.tile([64, W], BF16, tag="dat", name="dat")
                    nc.vector.tensor_copy(den[:], ot[par][64:65, :])
                    nc.vector.tensor_copy(dat[:], ot[par][0:64, :])
                    st[par] = (den, dat)
                staged[(j, th)] = st
                pending_norm.append((j, th))

            def finish_norms():
                while pending_norm:
                    j, th = pending_norm.pop(0)
                    st = staged.pop((j, th))
                    for par in range(2):
                        den, dat = st[par]
                        rt = small.tile([1, W], F32, tag="rt", name="rt")
                        Rt = small.tile([64, W], F32, tag="Rt", name="Rt")
                        # NOTE: reciprocal input must be partition-aligned
                        # with its output (partition-shifted non-copy DVE
                        # ops silently corrupt); the den copy realigns.
                        nc.vector.reciprocal_approx_fast(rt[:], den[:])
                        nc.gpsimd.partition_broadcast(Rt[:], rt[:])
                        nc.vector.tensor_mul(
                            attnT[j][par * 64 : (par + 1) * 64, th * W : (th + 1) * W],
                            dat[:],
                            Rt[:],
                        )

            def window(j, th, wl=None, inw=False, adds=(), per_slot=1,
                       inw_banks="o"):
                # norms first: attnT writes must be issued before any
                # freshly-added o_pair readers (issue order = dep order).
                # wl = window-lagged PV of a previous (j', th') on o banks;
                # inw = this window's own PV, lagged 2 slots.
                finish_norms()
                stream.extend(adds)
                # 2-slot score lookahead: S(i+2) issues right after ACT(i)
                score_mm(j, th, 0)
                score_mm(j, th, 1)
                for i in range(TP):
                    act_exp(j, th, i)
                    if i + 2 < TP:
                        score_mm(j, th, i + 2)
                    if wl is not None:
                        pv(wl[0], wl[1], i)
                    if inw and i >= 2:
                        pv(j, th, i - 2, banks=inw_banks)
                    pump(per_slot)
                if wl is not None:
                    stage_o(*wl)
                if inw:
                    pv(j, th, TP - 2, banks=inw_banks)
                    pv(j, th, TP - 1, banks=inw_banks)
                    stage_o(j, th)

            # ---------- schedule ----------
            # PE warmup during the input-DMA wait: ~18 matmuls on scratch
            # keep the tensor engine continuously busy so it reaches max
            # pstate before the real prologue (cold-start matmuls run ~2x
            # slow otherwise); result is never read.
            nc.vector.memset(scr[:], 0.5)
            wps = ps_p.tile([128, W], F32, tag="p", name="warm")
            for w in range(18):
                nc.tensor.matmul(
                    wps[:],
                    scr[0:128, 0:128],
                    scr[:],
                    start=(w == 0),
                    stop=(w == 17),
                )
            # prologue: k tb0 fully (chases xt0 arriving behind wk), then
            # q tb0 (wq lands during the k matmuls)
            for g in (qk_group(0, kT[0], wkt, 0), qk_group(0, qT[0], wqt, 0)):
                for _ in g:
                    pass

            window(
                0, 0, None,
                adds=[
                    qk_group(0, kT[0], wkt, 1),
                    qk_group(0, kT[0], wkt, 2),
                    qk_group(0, kT[0], wkt, 3),
                    qk_group(0, qT[0], wqt, 1),
                ]
                + [v_group(tp) for tp in range(TP)],
                per_slot=3,
            )
            window(
                0, 1, (0, 0),
                adds=[
                    qk_group(0, qT[0], wqt, 2),
                    qk_group(0, qT[0], wqt, 3),
                    qk_group(1, kT[1], wkt, 0),
                    qk_group(1, kT[1], wkt, 1),
                ],
            )
            window(
                0, 2, (0, 1),
                adds=[
                    qk_group(1, kT[1], wkt, 2),
                    qk_group(1, kT[1], wkt, 3),
                    qk_group(1, qT[1], wqt, 0),
                    qk_group(1, qT[1], wqt, 1),
                ],
            )
            window(0, 3, (0, 2), adds=[qk_group(1, qT[1], wqt, 2)])

            # transition window: PV(0,3) window-lagged on o banks + own
            # PV(1,0) in-window on p banks (stream is empty here)
            window(1, 0, (0, 3), inw=True, inw_banks="p")
            window(1, 1, inw=True, adds=[qk_group(1, qT[1], wqt, 3)])
            window(1, 2, inw=True, adds=[o_pair(tp) for tp in range(0, 4)])
            window(1, 3, inw=True, adds=[o_pair(tp) for tp in range(4, 9)])

            # tail: leftover th2 o_pairs drain while the (1,3) norm chain
            # runs on vector/gpsimd; then th3 o_pairs rotate over 4 psum
            # banks with casts split scalar/vector (scalar idle post-EXP)
            stream.extend([o_pair(9), o_pair(10), o_pair(11)])
            pump(6)
            finish_norms()
            stream.extend(
                o_pair(
                    tp,
                    tags=((None, None) if tp % 2 == 0 else ("oA", "oB")),
                    scalar_cast=True,
                )
                for tp in range(12, 16)
            )
            drain_stream()

    nc.compile()
    return nc


def kernel(x, wq, wk, wv, wo, trace=False):
    global _cached_nc
    if _cached_nc is None:
        _cached_nc = _build()
    nc = _cached_nc

    x = np.asarray(x, dtype=np.float32)
    wq = np.asarray(wq, dtype=np.float32)
    wk = np.asarray(wk, dtype=np.float32)
    wv = np.asarray(wv, dtype=np.float32)
    wo = np.asarray(wo, dtype=np.float32)

    in_maps = []
    for c in range(8):
        b, g = c // 4, c % 4
        cs = slice(g * CH, (g + 1) * CH)
        xb = np.ascontiguousarray(x[b].T)
        im = {
            "wq": _wlayout(wq[:, cs]).astype(ml_dtypes.bfloat16),
            "wk": _wlayout(wk[:, cs]).astype(ml_dtypes.bfloat16),
            "wv": _wlayout(wv[:, cs]).astype(ml_dtypes.bfloat16),
            "wo": _wlayout(wo[cs, :]).astype(ml_dtypes.bfloat16),
        }
        for tb in range(NTH):
            im[f"xt{tb}"] = _wlayout(xb[:, tb * W : (tb + 1) * W]).astype(
                ml_dtypes.bfloat16
            )
        in_maps.append(im)

    # the device intermittently drops input DMAs after a prior crash,
    # yielding inf/garbage; detect the signature and retry (healthy runs
    # have |y| ~ O(1))
    for _attempt in range(4):
        res = run_bass_kernel_spmd(
            nc, in_maps, core_ids=list(range(8)), trace=trace
        )
        out = np.zeros((B, T, D), np.float32)
        for c in range(8):
            b = c // 4
            out[b] += res.results[c]["y"].astype(np.float32)
        if np.isfinite(out).all() and np.abs(out).max() < 1e3:
            break
    if trace:
        kernel.last_results = res
    return out
